# revision 84
# baseline (speedup 1.0000x reference)
"""DeepLatent loss kernel for Trainium2 (8 NeuronCores, data-parallel over batch).

Computes, for pc/pc_gt [64,3,1024], latent [64,64] and a 3-layer per-point MLP:
    noise  = MLP(concat(pc, latent))          (per-point 1x1 convs)
    pc_est = pc - noise
    loss_chamfer = bidirectional mean-min-sqdist(pc_gt, pc_est)
    loss_L2      = mean((pc_gt - pc_est)**2)
    loss = 0.1*loss_chamfer + 0.9*loss_L2
Returns (loss, loss_chamfer, loss_L2, pc_est) like the reference.

Sharding: batch dim 64 -> 8 cores x 8 clouds. Each core runs the full
MLP + chamfer for its 8 clouds; scalar partial sums are combined on host.

Key design points:
- All matmuls stream as float32r (TF32-class, 4x the fp32 rate). bf16 would
  be fatal for the distance matrix (|gt|^2+|est|^2-2gt.est cancels ~6-scale
  terms down to ~0.05 minima).
- The distance matrix is ONE K=9 matmul per 128-row tile: rows pair
  (gt, -2est), (gt^2, ones), (ones, est^2) so the PE also performs the
  |gt|^2 / |est|^2 summations.
- Per tile, a custom DVE op (COPY_MIN) turns the PSUM tile into a bf16
  SBUF copy AND the per-partition row-min in a single 1x pass; the
  opposite-direction running min then runs on bf16 SBUF pairs at DVE 2x.
- The final cross-partition min uses PE transposes + one DVE reduce.
- Emission is software-pipelined: batch b+1's MLP stages are emitted
  between batch b's chamfer tiles so the static per-engine schedule
  interleaves them.
"""

import sys

sys.path.insert(0, "/opt/trn_rl_repo")

import numpy as np

import concourse.bacc as bacc
import concourse.tile as tile
from concourse import dve_ops, mybir
from concourse.bass_utils import run_bass_kernel_spmd
from concourse.dve_spec import C0, Spec, Src0, lower, minn
from concourse.dve_uop import DveOpSpec

NB = 64          # total batch
NPTS = 1024      # points per cloud
LAT = 64         # latent dim
HID = 128        # MLP hidden
NCORES = 8
BPC = NB // NCORES          # clouds per core
FN = BPC * NPTS             # free-dim elements per core (8192)
CHAMFER_WEIGHT = 0.1

F32 = mybir.dt.float32
F32R = mybir.dt.float32r
BF16 = mybir.dt.bfloat16
AX = mybir.AxisListType
OP = mybir.AluOpType
ACTFN = mybir.ActivationFunctionType

_PROGRAM_CACHE = {}


def _register_copy_min():
    """Custom DVE op: out = copy(in0) (with dtype cast), accum_out = min(in0).

    One DVE pass turns a PSUM fp32 distance tile into a bf16 SBUF copy AND
    the per-partition row-min — replacing a reduce + copy pair. The micro-op
    program is packed into the per-NEFF DVE table at compile time.
    """
    name = "COPY_MIN_ANT"
    if name in dve_ops._SUB_OPCODE_FOR_NAME:
        for op in dve_ops.OPS:
            if op.name == name:
                return op

    def _ref(in0, in1, c0, c1, c2):
        b = np.asarray(in0, np.float32)
        return b, np.minimum(c0, b.reshape(b.shape[0], -1).min(axis=-1, keepdims=True))

    spec = Spec(body=Src0, accum=minn, accum_init=C0, reference=_ref)  # s0= +huge
    op = dve_ops.DveOp(name, spec, subdim=False, uops_sha={})
    dve_ops.OPS.append(op)
    dve_ops._SUB_OPCODE_FOR_NAME[name] = max(dve_ops._SUB_OPCODE_FOR_NAME.values()) + 1
    assert dve_ops._SUB_OPCODE_FOR_NAME[name] < 0x20
    dve_ops.CUSTOM_DVE_SPECS[name] = spec
    for ver in ("v3", "v4"):
        compiled = DveOpSpec(
            name=name,
            opcode=dve_ops.get_dve_sub_opcode(name),
            uops=lower(spec, ver=ver),
            rd1_en=False,
        )
        op.uops_sha[ver] = compiled.sha(ver)
    return op


COPY_MIN = _register_copy_min()


def build_program():
    """Build the per-core Bass/Tile program (same NEFF on all 8 cores)."""
    nc = bacc.Bacc("TRN2", target_bir_lowering=False, debug=False)

    # ---- I/O --------------------------------------------------------------
    # pgT:    rows 0-2 pc, 3-5 gt, 6 ones                      [7, FN]
    # gaT:    rows 0-2 gt, 3-5 zeros(->gt^2 via hop), 6-8 ones [9, FN]
    # eauginit: rows 0-2 any(->-2est), 3-5 ones, 6-8 any(->est^2)
    pgT = nc.declare_dram_parameter("pgT", [7, FN], F32R, isOutput=False)
    gaT = nc.declare_dram_parameter("gaT", [9, FN], F32R, isOutput=False)
    eauginit = nc.declare_dram_parameter("eauginit", [3, FN], F32R, isOutput=False)
    latT = nc.declare_dram_parameter("latT", [LAT, BPC], F32R, isOutput=False)
    w1pT = nc.declare_dram_parameter("w1pT", [3, HID], F32R, isOutput=False)
    w1lT = nc.declare_dram_parameter("w1lT", [LAT, HID], F32R, isOutput=False)
    b1c = nc.declare_dram_parameter("b1c", [HID, 1], F32, isOutput=False)
    w2T = nc.declare_dram_parameter("w2T", [HID, HID], F32R, isOutput=False)
    b2c = nc.declare_dram_parameter("b2c", [HID, 1], F32, isOutput=False)
    w3n6 = nc.declare_dram_parameter("w3n6", [HID, 36], F32R, isOutput=False)
    pgl = nc.declare_dram_parameter("pgl", [7, 36], F32R, isOutput=False)
    identin = nc.declare_dram_parameter("identin", [HID, HID], BF16, isOutput=False)
    ones128in = nc.declare_dram_parameter("ones128in", [HID, 1], F32, isOutput=False)
    zeros4in = nc.declare_dram_parameter("zeros4in", [HID, 4], F32, isOutput=False)

    est_out = nc.declare_dram_parameter("est", [3, FN], F32, isOutput=True)
    partials = nc.declare_dram_parameter("partials", [4, 1], F32, isOutput=True)

    with tile.TileContext(nc) as tc:
        with (
            tc.tile_pool(name="singles", bufs=1) as singles,
            tc.tile_pool(name="hpool", bufs=2) as hpool,
            tc.tile_pool(name="rpool", bufs=4) as rpool,
            tc.tile_pool(name="jpool", bufs=2) as jpool,
            tc.tile_pool(name="spool", bufs=2) as spool,
            tc.tile_pool(name="mmp", bufs=2, space="PSUM") as mmp,
            tc.tile_pool(name="dps", bufs=2, space="PSUM") as dps,
        ):
            # ---- constants / big persistent tiles -------------------------
            # (pg/ga/eaug are DMA'd in per-cloud slices so cloud 0's MLP and
            # chamfer deps clear early instead of waiting on 800KB of DMA)
            pg = singles.tile([7, FN], F32R)
            ga = singles.tile([9, FN], F32R)
            eaug = singles.tile([9, FN], F32R)
            est_sb = singles.tile([3, FN], F32)

            lat_sb = singles.tile([LAT, BPC], F32R)
            nc.sync.dma_start(out=lat_sb[:], in_=latT[:])
            w1p_sb = singles.tile([3, HID], F32R)
            nc.sync.dma_start(out=w1p_sb[:], in_=w1pT[:])
            w1l_sb = singles.tile([LAT, HID], F32R)
            nc.sync.dma_start(out=w1l_sb[:], in_=w1lT[:])
            b1_sb = singles.tile([HID, 1], F32)
            nc.sync.dma_start(out=b1_sb[:], in_=b1c[:])
            w2_sb = singles.tile([HID, HID], F32R)
            nc.sync.dma_start(out=w2_sb[:], in_=w2T[:])
            b2_sb = singles.tile([HID, 1], F32)
            nc.sync.dma_start(out=b2_sb[:], in_=b2c[:])
            w3_sb = singles.tile([HID, 36], F32R)
            nc.sync.dma_start(out=w3_sb[:], in_=w3n6[:])
            pgl_sb = singles.tile([7, 36], F32R)
            nc.sync.dma_start(out=pgl_sb[:], in_=pgl[:])
            ident = singles.tile([HID, HID], BF16)
            nc.sync.dma_start(out=ident[:], in_=identin[:])
            ones128 = singles.tile([HID, 1], F32)
            nc.sync.dma_start(out=ones128[:], in_=ones128in[:])

            # cloud 0's slices first so its MLP/chamfer deps clear early;
            # ga rows 3-5 and eaug rows 0-2/6-8 are device-written before
            # any read, so only the host-meaningful rows are DMA'd.
            s0 = slice(0, NPTS)
            with tc.high_priority():
                nc.sync.dma_start(out=pg[:, s0], in_=pgT[:, s0])
                nc.sync.dma_start(out=ga[0:3, s0], in_=gaT[0:3, s0])
                nc.sync.dma_start(out=ga[6:9, s0], in_=gaT[6:9, s0])
                nc.sync.dma_start(out=eaug[3:6, s0], in_=eauginit[:, s0])
            rs = slice(NPTS, FN)
            nc.sync.dma_start(out=pg[:, rs], in_=pgT[:, rs])
            nc.sync.dma_start(out=ga[0:3, rs], in_=gaT[0:3, rs])
            nc.sync.dma_start(out=ga[6:9, rs], in_=gaT[6:9, rs])
            nc.sync.dma_start(out=eaug[3:6, rs], in_=eauginit[:, rs])

            dirA = singles.tile([HID, BPC], F32)    # per-batch row-min sums
            dirB = singles.tile([HID, BPC], F32)    # per-batch col-min sums
            l2acc = singles.tile([3, BPC], F32)     # per-batch L2 partial sums

            # ---- W1 latent term: bias1[:, b] = W1[:,3:] @ latent[b] + b1 --
            w1lat_ps = mmp.tile([HID, NPTS], F32, tag="mm")
            nc.tensor.matmul(
                w1lat_ps[:, 0:BPC], w1l_sb[:], lat_sb[:], start=True, stop=True
            )
            bias1 = singles.tile([HID, BPC], F32)
            nc.vector.tensor_scalar_add(bias1[:], w1lat_ps[:, 0:BPC], b1_sb[:])

            # ---- per-cloud stage emitters ---------------------------------
            def emit_h1(b):
                b0 = b * NPTS
                h1_ps = mmp.tile([HID, NPTS], F32, tag="mm")
                nc.tensor.matmul(
                    h1_ps[:, 0:512], w1p_sb[:], pg[0:3, b0 : b0 + 512],
                    start=True, stop=True,
                )
                nc.tensor.matmul(
                    h1_ps[:, 512:NPTS], w1p_sb[:], pg[0:3, b0 + 512 : b0 + NPTS],
                    start=True, stop=True,
                )
                h1 = hpool.tile([HID, NPTS], F32R, tag="h1")
                with tc.high_priority(offset=48):
                    nc.scalar.activation(
                        h1[:], h1_ps[:], ACTFN.Relu, bias=bias1[:, b : b + 1], scale=1.0
                    )
                return h1

            def emit_h2(b, h1):
                h2_ps = mmp.tile([HID, NPTS], F32, tag="mm")
                nc.tensor.matmul(
                    h2_ps[:, 0:512], w2_sb[:], h1[:, 0:512], start=True, stop=True
                )
                nc.tensor.matmul(
                    h2_ps[:, 512:NPTS], w2_sb[:], h1[:, 512:NPTS], start=True, stop=True
                )
                h2 = hpool.tile([HID, NPTS], F32R, tag="h2")
                with tc.high_priority(offset=48):
                    nc.scalar.activation(
                        h2[:], h2_ps[:], ACTFN.Relu, bias=b2_sb[:], scale=1.0
                    )
                return h2

            def emit_est_mm(b, h2, half, ed_ps=None):
                # est/diff: psum rows 0-2 = pc_est, rows 32-34 = gt - pc_est
                # (engine APs may only start at partitions 0/32/64/96).
                # Halves are emitted separately so the PE work spreads over
                # two chamfer-tile gaps instead of blocking one d fill.
                b0 = b * NPTS
                if ed_ps is None:
                    ed_ps = mmp.tile([36, NPTS], F32, tag="mm")
                lo, hi = half * 512, (half + 1) * 512
                nc.tensor.matmul(
                    ed_ps[:, lo:hi], w3_sb[:], h2[:, lo:hi], start=True, stop=False
                )
                nc.tensor.matmul(
                    ed_ps[:, lo:hi], pgl_sb[:], pg[:, b0 + lo : b0 + hi],
                    start=False, stop=True,
                )
                return ed_ps

            def emit_est_tail(b, ed_ps):
                b0 = b * NPTS
                bsl = slice(b0, b0 + NPTS)
                # chamfer-critical ops first (the scheduler otherwise parks
                # them behind the next cloud's ReLUs, delaying the d matmuls)
                with tc.high_priority(offset=64):
                    nc.scalar.mul(eaug[0:3, bsl], ed_ps[0:3, :], -2.0)
                    esq = jpool.tile([3, NPTS], F32R, tag="esq")
                    nc.scalar.activation(esq[:], ed_ps[0:3, :], ACTFN.Square)
                    # SBUF->SBUF DMA hop: engines cannot write partitions 6-8
                    nc.sync.dma_start(out=eaug[6:9, bsl], in_=esq[:])
                nc.scalar.copy(est_sb[:, bsl], ed_ps[0:3, :])
                junk = jpool.tile([3, NPTS], F32, tag="junk")
                nc.scalar.activation(
                    junk[:], ed_ps[32:35, :], ACTFN.Square,
                    accum_out=l2acc[:, b : b + 1],
                )
                nc.sync.dma_start(out=est_out[:, bsl], in_=est_sb[:, bsl])

            def emit_gtprep(b):
                # gt^2 rows for the augmented lhsT (rows 3-5 of ga, via hop)
                b0 = b * NPTS
                with tc.high_priority(offset=64):
                    gtsq = jpool.tile([3, NPTS], F32R, tag="gtsq")
                    nc.scalar.activation(
                        gtsq[:], ga[0:3, b0 : b0 + NPTS], ACTFN.Square
                    )
                    nc.sync.dma_start(out=ga[3:6, b0 : b0 + NPTS], in_=gtsq[:])

            def emit_mlp(b):
                h1 = emit_h1(b)
                h2 = emit_h2(b, h1)
                ed = emit_est_mm(b, h2, 0)
                emit_est_mm(b, h2, 1, ed)
                emit_est_tail(b, ed)
                emit_gtprep(b)

            def emit_colmin(b, rmin_final, rowmins):
                # col-min: transpose rmin 128x128 chunks, reduce over partitions
                t_ps = mmp.tile([HID, 8, HID], BF16, tag="mm")
                for u in range(8):
                    nc.tensor.transpose(
                        t_ps[:, u, :], rmin_final[:, u * HID : (u + 1) * HID], ident[:]
                    )
                colmins = spool.tile([HID, 8], F32, tag="colmins")
                nc.vector.tensor_reduce(
                    out=colmins[:], in_=t_ps[:], axis=AX.X, op=OP.min
                )
                nc.vector.tensor_reduce(
                    out=dirA[:, b : b + 1], in_=rowmins[:], axis=AX.X, op=OP.add
                )
                nc.vector.tensor_reduce(
                    out=dirB[:, b : b + 1], in_=colmins[:], axis=AX.X, op=OP.add
                )

            # ---- prologue: cloud 0's MLP + cloud 1's layer 1 --------------
            emit_mlp(0)
            h1_cache = {}
            if BPC > 1:
                h1_cache[1] = emit_h1(1)

            # ---- main software-pipelined loop -----------------------------
            pending = None  # (b, rmin_final, rowmins) awaiting colmin
            for b in range(BPC):
                b0 = b * NPTS
                rowmins = spool.tile([HID, 8], F32, tag="rowmins")
                rmin_prev = None
                nb = b + 1
                for t in range(8):
                    d_ps = dps.tile([HID, NPTS], F32, tag="d")
                    off = b0 + t * HID
                    nc.tensor.matmul(
                        d_ps[:, 0:512], ga[:, off : off + HID],
                        eaug[:, b0 : b0 + 512], start=True, stop=True,
                    )
                    nc.tensor.matmul(
                        d_ps[:, 512:NPTS], ga[:, off : off + HID],
                        eaug[:, b0 + 512 : b0 + NPTS], start=True, stop=True,
                    )
                    d_sb = rpool.tile([HID, NPTS], BF16, tag="dsb")
                    nc.vector._custom_dve(
                        COPY_MIN,
                        out=d_sb[:],
                        in0=d_ps[:],
                        s0=3.0e38,
                        accum_out=rowmins[:, t : t + 1],
                    )
                    if t == 0:
                        rmin_prev = d_sb
                        # previous cloud's colmin, after this cloud's first
                        # tile is in flight (keeps PE on d tiles, no DVE gap)
                        if pending is not None:
                            emit_colmin(*pending)
                            pending = None
                    else:
                        rmin = rpool.tile([HID, NPTS], BF16, tag="rmin")
                        nc.vector.tensor_tensor(
                            out=rmin[:], in0=d_sb[:], in1=rmin_prev[:], op=OP.min
                        )
                        rmin_prev = rmin
                    # two-batch-deep pipeline: every MLP stage of cloud
                    # b+1 has a full batch of lead over its in-order PE slot,
                    # so nothing stalls waiting on a ReLU semaphore
                    if nb < BPC:
                        if t == 0:
                            h2_next = emit_h2(nb, h1_cache.pop(nb))
                        elif t == 2:
                            ed_next = emit_est_mm(nb, h2_next, 0)
                        elif t == 3:
                            emit_est_mm(nb, h2_next, 1, ed_next)
                            emit_est_tail(nb, ed_next)
                        elif t == 5:
                            emit_gtprep(nb)
                        elif t == 6 and nb + 1 < BPC:
                            h1_cache[nb + 1] = emit_h1(nb + 1)
                pending = (b, rmin_prev, rowmins)
            emit_colmin(*pending)

            # ---- final partial sums (cross-partition via ones matmul) -----
            fin = singles.tile([HID, 4], F32)
            nc.sync.dma_start(out=fin[:], in_=zeros4in[:])
            nc.vector.tensor_reduce(out=fin[:, 0:1], in_=dirA[:], axis=AX.X, op=OP.add)
            nc.vector.tensor_reduce(out=fin[:, 1:2], in_=dirB[:], axis=AX.X, op=OP.add)
            nc.vector.tensor_reduce(
                out=fin[0:3, 2:3], in_=l2acc[:], axis=AX.X, op=OP.add
            )
            pp = mmp.tile([4, 1], F32, tag="mm")
            nc.tensor.matmul(pp[:], fin[:], ones128[:], start=True, stop=True)
            outp = singles.tile([4, 1], F32)
            nc.scalar.copy(outp[:], pp[:])
            nc.sync.dma_start(out=partials[:], in_=outp[:])

    nc.finalize()
    return nc


def get_program():
    if "nc" not in _PROGRAM_CACHE:
        _PROGRAM_CACHE["nc"] = build_program()
    return _PROGRAM_CACHE["nc"]


def make_in_maps(pc, pc_gt, latent, W1, b1, W2, b2, W3, b3):
    import ml_dtypes

    pc = np.ascontiguousarray(pc, dtype=np.float32)
    pc_gt = np.ascontiguousarray(pc_gt, dtype=np.float32)
    latent = np.ascontiguousarray(latent, dtype=np.float32)
    W1 = np.asarray(W1, dtype=np.float32)
    b1 = np.asarray(b1, dtype=np.float32)
    W2 = np.asarray(W2, dtype=np.float32)
    b2 = np.asarray(b2, dtype=np.float32)
    W3 = np.asarray(W3, dtype=np.float32)
    b3 = np.asarray(b3, dtype=np.float32)

    eye3 = np.eye(3, dtype=np.float32)
    pgl = np.zeros((7, 36), np.float32)
    pgl[0:3, 0:3] = eye3        # est rows: +pc
    pgl[0:3, 32:35] = -eye3     # diff rows: -pc
    pgl[3:6, 32:35] = eye3      # diff rows: +gt
    pgl[6, 0:3] = -b3           # est rows: -b3
    pgl[6, 32:35] = b3          # diff rows: +b3
    w3n36 = np.zeros((128, 36), np.float32)
    w3n36[:, 0:3] = -W3.T       # est rows: -(W3 @ h2)
    w3n36[:, 32:35] = W3.T      # diff rows: +(W3 @ h2)
    consts = {
        "w1pT": np.ascontiguousarray(W1[:, :3].T),
        "w1lT": np.ascontiguousarray(W1[:, 3:].T),
        "b1c": b1[:, None].copy(),
        "w2T": np.ascontiguousarray(W2.T),
        "b2c": b2[:, None].copy(),
        "w3n6": w3n36,
        "pgl": pgl,
        "identin": np.eye(HID, dtype=ml_dtypes.bfloat16),
        "ones128in": np.ones((HID, 1), np.float32),
        "zeros4in": np.zeros((HID, 4), np.float32),
    }

    ones1 = np.ones((1, FN), np.float32)
    eauginit = np.ones((3, FN), np.float32)
    in_maps = []
    for i in range(NCORES):
        sl = slice(i * BPC, (i + 1) * BPC)
        pcT = pc[sl].transpose(1, 0, 2).reshape(3, FN)
        gtT = pc_gt[sl].transpose(1, 0, 2).reshape(3, FN)
        pgT = np.concatenate([pcT, gtT, ones1], axis=0)
        gaT = np.concatenate(
            [gtT, np.zeros((3, FN), np.float32), np.ones((3, FN), np.float32)],
            axis=0,
        )
        latTc = np.ascontiguousarray(latent[sl].T)
        in_maps.append(
            {
                "pgT": np.ascontiguousarray(pgT),
                "gaT": np.ascontiguousarray(gaT),
                "eauginit": eauginit,
                "latT": latTc,
                **consts,
            }
        )
    return in_maps


def combine_outputs(results):
    """results: list (per core) of {"est": [3, FN], "partials": [4, 1]}."""
    est_parts = []
    sums = np.zeros(4, dtype=np.float64)
    for r in results:
        est_parts.append(
            np.asarray(r["est"]).reshape(3, BPC, NPTS).transpose(1, 0, 2)
        )
        sums += np.asarray(r["partials"], dtype=np.float64)[:, 0]
    pc_est = np.concatenate(est_parts, axis=0).astype(np.float32)
    loss_ch = sums[0] / (NB * NPTS) + sums[1] / (NB * NPTS)
    loss_l2 = sums[2] / (NB * 3 * NPTS)
    loss = CHAMFER_WEIGHT * loss_ch + (1.0 - CHAMFER_WEIGHT) * loss_l2
    return (
        np.float32(loss),
        np.float32(loss_ch),
        np.float32(loss_l2),
        pc_est,
    )


def run_sharded(inputs, trace=False, **kw):
    nc = get_program()
    in_maps = make_in_maps(**inputs)
    res = run_bass_kernel_spmd(
        nc, in_maps, core_ids=list(range(NCORES)), trace=trace, **kw
    )
    return combine_outputs(res.results), res


def kernel(**inputs):
    out, _ = run_sharded(inputs, trace=False)
    return out


# revision 86
# speedup vs baseline: 1.0047x; 1.0047x over previous
"""DeepLatent loss kernel for Trainium2 (8 NeuronCores, data-parallel over batch).

Computes, for pc/pc_gt [64,3,1024], latent [64,64] and a 3-layer per-point MLP:
    noise  = MLP(concat(pc, latent))          (per-point 1x1 convs)
    pc_est = pc - noise
    loss_chamfer = bidirectional mean-min-sqdist(pc_gt, pc_est)
    loss_L2      = mean((pc_gt - pc_est)**2)
    loss = 0.1*loss_chamfer + 0.9*loss_L2
Returns (loss, loss_chamfer, loss_L2, pc_est) like the reference.

Sharding: batch dim 64 -> 8 cores x 8 clouds. Each core runs the full
MLP + chamfer for its 8 clouds; scalar partial sums are combined on host.

Key design points:
- All matmuls stream as float32r (TF32-class, 4x the fp32 rate). bf16 would
  be fatal for the distance matrix (|gt|^2+|est|^2-2gt.est cancels ~6-scale
  terms down to ~0.05 minima).
- The distance matrix is ONE K=9 matmul per 128-row tile: rows pair
  (gt, -2est), (gt^2, ones), (ones, est^2) so the PE also performs the
  |gt|^2 / |est|^2 summations.
- Per tile, a custom DVE op (COPY_MIN) turns the PSUM tile into a bf16
  SBUF copy AND the per-partition row-min in a single 1x pass; the
  opposite-direction running min then runs on bf16 SBUF pairs at DVE 2x.
- The final cross-partition min uses PE transposes + one DVE reduce.
- Emission is software-pipelined: batch b+1's MLP stages are emitted
  between batch b's chamfer tiles so the static per-engine schedule
  interleaves them.
"""

import sys

sys.path.insert(0, "/opt/trn_rl_repo")

import numpy as np

import concourse.bacc as bacc
import concourse.tile as tile
from concourse import dve_ops, mybir
from concourse.bass_utils import run_bass_kernel_spmd
from concourse.dve_spec import C0, Spec, Src0, lower, minn
from concourse.dve_uop import DveOpSpec

NB = 64          # total batch
NPTS = 1024      # points per cloud
LAT = 64         # latent dim
HID = 128        # MLP hidden
NCORES = 8
BPC = NB // NCORES          # clouds per core
FN = BPC * NPTS             # free-dim elements per core (8192)
CHAMFER_WEIGHT = 0.1

F32 = mybir.dt.float32
F32R = mybir.dt.float32r
BF16 = mybir.dt.bfloat16
AX = mybir.AxisListType
OP = mybir.AluOpType
ACTFN = mybir.ActivationFunctionType

_PROGRAM_CACHE = {}


def _register_copy_min():
    """Custom DVE op: out = copy(in0) (with dtype cast), accum_out = min(in0).

    One DVE pass turns a PSUM fp32 distance tile into a bf16 SBUF copy AND
    the per-partition row-min — replacing a reduce + copy pair. The micro-op
    program is packed into the per-NEFF DVE table at compile time.
    """
    name = "COPY_MIN_ANT"
    if name in dve_ops._SUB_OPCODE_FOR_NAME:
        for op in dve_ops.OPS:
            if op.name == name:
                return op

    def _ref(in0, in1, c0, c1, c2):
        b = np.asarray(in0, np.float32)
        return b, np.minimum(c0, b.reshape(b.shape[0], -1).min(axis=-1, keepdims=True))

    spec = Spec(body=Src0, accum=minn, accum_init=C0, reference=_ref)  # s0= +huge
    op = dve_ops.DveOp(name, spec, subdim=False, uops_sha={})
    dve_ops.OPS.append(op)
    dve_ops._SUB_OPCODE_FOR_NAME[name] = max(dve_ops._SUB_OPCODE_FOR_NAME.values()) + 1
    assert dve_ops._SUB_OPCODE_FOR_NAME[name] < 0x20
    dve_ops.CUSTOM_DVE_SPECS[name] = spec
    for ver in ("v3", "v4"):
        compiled = DveOpSpec(
            name=name,
            opcode=dve_ops.get_dve_sub_opcode(name),
            uops=lower(spec, ver=ver),
            rd1_en=False,
        )
        op.uops_sha[ver] = compiled.sha(ver)
    return op


COPY_MIN = _register_copy_min()


def build_program():
    """Build the per-core Bass/Tile program (same NEFF on all 8 cores)."""
    nc = bacc.Bacc("TRN2", target_bir_lowering=False, debug=False)

    # ---- I/O --------------------------------------------------------------
    # pgT:    rows 0-2 pc, 3-5 gt, 6 ones                      [7, FN]
    # gaT:    rows 0-2 gt, 3-5 zeros(->gt^2 via hop), 6-8 ones [9, FN]
    # eauginit: rows 0-2 any(->-2est), 3-5 ones, 6-8 any(->est^2)
    pgT = nc.declare_dram_parameter("pgT", [7, FN], F32R, isOutput=False)
    gaT = nc.declare_dram_parameter("gaT", [9, FN], F32R, isOutput=False)
    eauginit = nc.declare_dram_parameter("eauginit", [3, FN], F32R, isOutput=False)
    latT = nc.declare_dram_parameter("latT", [LAT, BPC], F32R, isOutput=False)
    w1pT = nc.declare_dram_parameter("w1pT", [3, HID], F32R, isOutput=False)
    w1lT = nc.declare_dram_parameter("w1lT", [LAT, HID], F32R, isOutput=False)
    b1c = nc.declare_dram_parameter("b1c", [HID, 1], F32, isOutput=False)
    w2T = nc.declare_dram_parameter("w2T", [HID, HID], F32R, isOutput=False)
    b2c = nc.declare_dram_parameter("b2c", [HID, 1], F32, isOutput=False)
    w3n6 = nc.declare_dram_parameter("w3n6", [HID, 36], F32R, isOutput=False)
    pgl = nc.declare_dram_parameter("pgl", [7, 36], F32R, isOutput=False)
    identin = nc.declare_dram_parameter("identin", [HID, HID], BF16, isOutput=False)
    ones128in = nc.declare_dram_parameter("ones128in", [HID, 1], F32, isOutput=False)
    zeros4in = nc.declare_dram_parameter("zeros4in", [HID, 4], F32, isOutput=False)

    est_out = nc.declare_dram_parameter("est", [3, FN], F32, isOutput=True)
    partials = nc.declare_dram_parameter("partials", [4, 1], F32, isOutput=True)

    with tile.TileContext(nc) as tc:
        with (
            tc.tile_pool(name="singles", bufs=1) as singles,
            tc.tile_pool(name="hpool", bufs=3) as hpool,
            tc.tile_pool(name="rpool", bufs=3) as rpool,
            tc.tile_pool(name="jpool", bufs=3) as jpool,
            tc.tile_pool(name="spool", bufs=3) as spool,
            tc.tile_pool(name="mmp", bufs=2, space="PSUM") as mmp,
            tc.tile_pool(name="dps", bufs=2, space="PSUM") as dps,
        ):
            # ---- constants / big persistent tiles -------------------------
            # (pg/ga/eaug are DMA'd in per-cloud slices so cloud 0's MLP and
            # chamfer deps clear early instead of waiting on 800KB of DMA)
            pg = singles.tile([7, FN], F32R)
            ga = singles.tile([9, FN], F32R)
            eaug = singles.tile([9, FN], F32R)
            est_sb = singles.tile([3, FN], F32)

            lat_sb = singles.tile([LAT, BPC], F32R)
            nc.sync.dma_start(out=lat_sb[:], in_=latT[:])
            w1p_sb = singles.tile([3, HID], F32R)
            nc.sync.dma_start(out=w1p_sb[:], in_=w1pT[:])
            w1l_sb = singles.tile([LAT, HID], F32R)
            nc.sync.dma_start(out=w1l_sb[:], in_=w1lT[:])
            b1_sb = singles.tile([HID, 1], F32)
            nc.sync.dma_start(out=b1_sb[:], in_=b1c[:])
            w2_sb = singles.tile([HID, HID], F32R)
            nc.sync.dma_start(out=w2_sb[:], in_=w2T[:])
            b2_sb = singles.tile([HID, 1], F32)
            nc.sync.dma_start(out=b2_sb[:], in_=b2c[:])
            w3_sb = singles.tile([HID, 36], F32R)
            nc.sync.dma_start(out=w3_sb[:], in_=w3n6[:])
            pgl_sb = singles.tile([7, 36], F32R)
            nc.sync.dma_start(out=pgl_sb[:], in_=pgl[:])
            ident = singles.tile([HID, HID], BF16)
            nc.sync.dma_start(out=ident[:], in_=identin[:])
            ones128 = singles.tile([HID, 1], F32)
            nc.sync.dma_start(out=ones128[:], in_=ones128in[:])

            # cloud 0's slices first so its MLP/chamfer deps clear early;
            # ga rows 3-5 and eaug rows 0-2/6-8 are device-written before
            # any read, so only the host-meaningful rows are DMA'd.
            s0 = slice(0, NPTS)
            with tc.high_priority():
                nc.sync.dma_start(out=pg[:, s0], in_=pgT[:, s0])
                nc.sync.dma_start(out=ga[0:3, s0], in_=gaT[0:3, s0])
                nc.sync.dma_start(out=ga[6:9, s0], in_=gaT[6:9, s0])
                nc.sync.dma_start(out=eaug[3:6, s0], in_=eauginit[:, s0])
            rs = slice(NPTS, FN)
            nc.sync.dma_start(out=pg[:, rs], in_=pgT[:, rs])
            nc.sync.dma_start(out=ga[0:3, rs], in_=gaT[0:3, rs])
            nc.sync.dma_start(out=ga[6:9, rs], in_=gaT[6:9, rs])
            nc.sync.dma_start(out=eaug[3:6, rs], in_=eauginit[:, rs])

            dirA = singles.tile([HID, BPC], F32)    # per-batch row-min sums
            dirB = singles.tile([HID, BPC], F32)    # per-batch col-min sums
            l2acc = singles.tile([3, BPC], F32)     # per-batch L2 partial sums

            # ---- W1 latent term: bias1[:, b] = W1[:,3:] @ latent[b] + b1 --
            w1lat_ps = mmp.tile([HID, NPTS], F32, tag="mm")
            nc.tensor.matmul(
                w1lat_ps[:, 0:BPC], w1l_sb[:], lat_sb[:], start=True, stop=True
            )
            bias1 = singles.tile([HID, BPC], F32)
            nc.vector.tensor_scalar_add(bias1[:], w1lat_ps[:, 0:BPC], b1_sb[:])

            # ---- per-cloud stage emitters ---------------------------------
            def emit_h1(b):
                b0 = b * NPTS
                h1_ps = mmp.tile([HID, NPTS], F32, tag="mm")
                nc.tensor.matmul(
                    h1_ps[:, 0:512], w1p_sb[:], pg[0:3, b0 : b0 + 512],
                    start=True, stop=True,
                )
                nc.tensor.matmul(
                    h1_ps[:, 512:NPTS], w1p_sb[:], pg[0:3, b0 + 512 : b0 + NPTS],
                    start=True, stop=True,
                )
                h1 = hpool.tile([HID, NPTS], F32R, tag="h1")
                with tc.high_priority(offset=48):
                    nc.scalar.activation(
                        h1[:], h1_ps[:], ACTFN.Relu, bias=bias1[:, b : b + 1], scale=1.0
                    )
                return h1

            def emit_h2(b, h1):
                h2_ps = mmp.tile([HID, NPTS], F32, tag="mm")
                nc.tensor.matmul(
                    h2_ps[:, 0:512], w2_sb[:], h1[:, 0:512], start=True, stop=True
                )
                nc.tensor.matmul(
                    h2_ps[:, 512:NPTS], w2_sb[:], h1[:, 512:NPTS], start=True, stop=True
                )
                h2 = hpool.tile([HID, NPTS], F32R, tag="h2")
                with tc.high_priority(offset=48):
                    nc.scalar.activation(
                        h2[:], h2_ps[:], ACTFN.Relu, bias=b2_sb[:], scale=1.0
                    )
                return h2

            def emit_est_mm(b, h2, half, ed_ps=None):
                # est/diff: psum rows 0-2 = pc_est, rows 32-34 = gt - pc_est
                # (engine APs may only start at partitions 0/32/64/96).
                # Halves are emitted separately so the PE work spreads over
                # two chamfer-tile gaps instead of blocking one d fill.
                b0 = b * NPTS
                if ed_ps is None:
                    ed_ps = mmp.tile([36, NPTS], F32, tag="mm")
                lo, hi = half * 512, (half + 1) * 512
                nc.tensor.matmul(
                    ed_ps[:, lo:hi], w3_sb[:], h2[:, lo:hi], start=True, stop=False
                )
                nc.tensor.matmul(
                    ed_ps[:, lo:hi], pgl_sb[:], pg[:, b0 + lo : b0 + hi],
                    start=False, stop=True,
                )
                return ed_ps

            def emit_est_tail(b, ed_ps):
                b0 = b * NPTS
                bsl = slice(b0, b0 + NPTS)
                # chamfer-critical ops first (the scheduler otherwise parks
                # them behind the next cloud's ReLUs, delaying the d matmuls)
                with tc.high_priority(offset=64):
                    nc.scalar.mul(eaug[0:3, bsl], ed_ps[0:3, :], -2.0)
                    esq = jpool.tile([3, NPTS], F32R, tag="esq")
                    nc.scalar.activation(esq[:], ed_ps[0:3, :], ACTFN.Square)
                    # SBUF->SBUF DMA hop: engines cannot write partitions 6-8
                    nc.sync.dma_start(out=eaug[6:9, bsl], in_=esq[:])
                nc.scalar.copy(est_sb[:, bsl], ed_ps[0:3, :])
                junk = jpool.tile([3, NPTS], F32, tag="junk")
                nc.scalar.activation(
                    junk[:], ed_ps[32:35, :], ACTFN.Square,
                    accum_out=l2acc[:, b : b + 1],
                )
                nc.sync.dma_start(out=est_out[:, bsl], in_=est_sb[:, bsl])

            def emit_gtprep(b):
                # gt^2 rows for the augmented lhsT (rows 3-5 of ga, via hop)
                b0 = b * NPTS
                with tc.high_priority(offset=64):
                    gtsq = jpool.tile([3, NPTS], F32R, tag="gtsq")
                    nc.scalar.activation(
                        gtsq[:], ga[0:3, b0 : b0 + NPTS], ACTFN.Square
                    )
                    nc.sync.dma_start(out=ga[3:6, b0 : b0 + NPTS], in_=gtsq[:])

            def emit_mlp(b):
                h1 = emit_h1(b)
                h2 = emit_h2(b, h1)
                ed = emit_est_mm(b, h2, 0)
                emit_est_mm(b, h2, 1, ed)
                emit_est_tail(b, ed)
                emit_gtprep(b)

            def emit_colmin(b, rmin_final, rowmins):
                # col-min: transpose rmin 128x128 chunks, reduce over partitions
                t_ps = mmp.tile([HID, 8, HID], BF16, tag="mm")
                for u in range(8):
                    nc.tensor.transpose(
                        t_ps[:, u, :], rmin_final[:, u * HID : (u + 1) * HID], ident[:]
                    )
                colmins = spool.tile([HID, 8], F32, tag="colmins")
                nc.vector.tensor_reduce(
                    out=colmins[:], in_=t_ps[:], axis=AX.X, op=OP.min
                )
                nc.vector.tensor_reduce(
                    out=dirA[:, b : b + 1], in_=rowmins[:], axis=AX.X, op=OP.add
                )
                nc.vector.tensor_reduce(
                    out=dirB[:, b : b + 1], in_=colmins[:], axis=AX.X, op=OP.add
                )

            # ---- prologue: cloud 0's MLP + cloud 1's layer 1 --------------
            emit_mlp(0)
            h1_cache = {}
            if BPC > 1:
                h1_cache[1] = emit_h1(1)

            # ---- main software-pipelined loop -----------------------------
            pending = None  # (b, rmin_final, rowmins) awaiting colmin
            for b in range(BPC):
                b0 = b * NPTS
                rowmins = spool.tile([HID, 8], F32, tag="rowmins")
                rmin_prev = None
                nb = b + 1
                for t in range(8):
                    d_ps = dps.tile([HID, NPTS], F32, tag="d")
                    off = b0 + t * HID
                    nc.tensor.matmul(
                        d_ps[:, 0:512], ga[:, off : off + HID],
                        eaug[:, b0 : b0 + 512], start=True, stop=True,
                    )
                    nc.tensor.matmul(
                        d_ps[:, 512:NPTS], ga[:, off : off + HID],
                        eaug[:, b0 + 512 : b0 + NPTS], start=True, stop=True,
                    )
                    d_sb = rpool.tile([HID, NPTS], BF16, tag="dsb")
                    nc.vector._custom_dve(
                        COPY_MIN,
                        out=d_sb[:],
                        in0=d_ps[:],
                        s0=3.0e38,
                        accum_out=rowmins[:, t : t + 1],
                    )
                    if t == 0:
                        rmin_prev = d_sb
                        # previous cloud's colmin, after this cloud's first
                        # tile is in flight (keeps PE on d tiles, no DVE gap)
                        if pending is not None:
                            emit_colmin(*pending)
                            pending = None
                    else:
                        rmin = rpool.tile([HID, NPTS], BF16, tag="rmin")
                        nc.vector.tensor_tensor(
                            out=rmin[:], in0=d_sb[:], in1=rmin_prev[:], op=OP.min
                        )
                        rmin_prev = rmin
                    # two-batch-deep pipeline: every MLP stage of cloud
                    # b+1 has a full batch of lead over its in-order PE slot,
                    # so nothing stalls waiting on a ReLU semaphore
                    if nb < BPC:
                        if t == 0:
                            h2_next = emit_h2(nb, h1_cache.pop(nb))
                        elif t == 2:
                            ed_next = emit_est_mm(nb, h2_next, 0)
                        elif t == 3:
                            emit_est_mm(nb, h2_next, 1, ed_next)
                            emit_est_tail(nb, ed_next)
                        elif t == 5:
                            emit_gtprep(nb)
                        elif t == 6 and nb + 1 < BPC:
                            h1_cache[nb + 1] = emit_h1(nb + 1)
                pending = (b, rmin_prev, rowmins)
            emit_colmin(*pending)

            # ---- final partial sums (cross-partition via ones matmul) -----
            fin = singles.tile([HID, 4], F32)
            nc.sync.dma_start(out=fin[:], in_=zeros4in[:])
            nc.vector.tensor_reduce(out=fin[:, 0:1], in_=dirA[:], axis=AX.X, op=OP.add)
            nc.vector.tensor_reduce(out=fin[:, 1:2], in_=dirB[:], axis=AX.X, op=OP.add)
            nc.vector.tensor_reduce(
                out=fin[0:3, 2:3], in_=l2acc[:], axis=AX.X, op=OP.add
            )
            pp = mmp.tile([4, 1], F32, tag="mm")
            nc.tensor.matmul(pp[:], fin[:], ones128[:], start=True, stop=True)
            outp = singles.tile([4, 1], F32)
            nc.scalar.copy(outp[:], pp[:])
            nc.sync.dma_start(out=partials[:], in_=outp[:])

    nc.finalize()
    return nc


def get_program():
    if "nc" not in _PROGRAM_CACHE:
        _PROGRAM_CACHE["nc"] = build_program()
    return _PROGRAM_CACHE["nc"]


def make_in_maps(pc, pc_gt, latent, W1, b1, W2, b2, W3, b3):
    import ml_dtypes

    pc = np.ascontiguousarray(pc, dtype=np.float32)
    pc_gt = np.ascontiguousarray(pc_gt, dtype=np.float32)
    latent = np.ascontiguousarray(latent, dtype=np.float32)
    W1 = np.asarray(W1, dtype=np.float32)
    b1 = np.asarray(b1, dtype=np.float32)
    W2 = np.asarray(W2, dtype=np.float32)
    b2 = np.asarray(b2, dtype=np.float32)
    W3 = np.asarray(W3, dtype=np.float32)
    b3 = np.asarray(b3, dtype=np.float32)

    eye3 = np.eye(3, dtype=np.float32)
    pgl = np.zeros((7, 36), np.float32)
    pgl[0:3, 0:3] = eye3        # est rows: +pc
    pgl[0:3, 32:35] = -eye3     # diff rows: -pc
    pgl[3:6, 32:35] = eye3      # diff rows: +gt
    pgl[6, 0:3] = -b3           # est rows: -b3
    pgl[6, 32:35] = b3          # diff rows: +b3
    w3n36 = np.zeros((128, 36), np.float32)
    w3n36[:, 0:3] = -W3.T       # est rows: -(W3 @ h2)
    w3n36[:, 32:35] = W3.T      # diff rows: +(W3 @ h2)
    consts = {
        "w1pT": np.ascontiguousarray(W1[:, :3].T),
        "w1lT": np.ascontiguousarray(W1[:, 3:].T),
        "b1c": b1[:, None].copy(),
        "w2T": np.ascontiguousarray(W2.T),
        "b2c": b2[:, None].copy(),
        "w3n6": w3n36,
        "pgl": pgl,
        "identin": np.eye(HID, dtype=ml_dtypes.bfloat16),
        "ones128in": np.ones((HID, 1), np.float32),
        "zeros4in": np.zeros((HID, 4), np.float32),
    }

    ones1 = np.ones((1, FN), np.float32)
    eauginit = np.ones((3, FN), np.float32)
    in_maps = []
    for i in range(NCORES):
        sl = slice(i * BPC, (i + 1) * BPC)
        pcT = pc[sl].transpose(1, 0, 2).reshape(3, FN)
        gtT = pc_gt[sl].transpose(1, 0, 2).reshape(3, FN)
        pgT = np.concatenate([pcT, gtT, ones1], axis=0)
        gaT = np.concatenate(
            [gtT, np.zeros((3, FN), np.float32), np.ones((3, FN), np.float32)],
            axis=0,
        )
        latTc = np.ascontiguousarray(latent[sl].T)
        in_maps.append(
            {
                "pgT": np.ascontiguousarray(pgT),
                "gaT": np.ascontiguousarray(gaT),
                "eauginit": eauginit,
                "latT": latTc,
                **consts,
            }
        )
    return in_maps


def combine_outputs(results):
    """results: list (per core) of {"est": [3, FN], "partials": [4, 1]}."""
    est_parts = []
    sums = np.zeros(4, dtype=np.float64)
    for r in results:
        est_parts.append(
            np.asarray(r["est"]).reshape(3, BPC, NPTS).transpose(1, 0, 2)
        )
        sums += np.asarray(r["partials"], dtype=np.float64)[:, 0]
    pc_est = np.concatenate(est_parts, axis=0).astype(np.float32)
    loss_ch = sums[0] / (NB * NPTS) + sums[1] / (NB * NPTS)
    loss_l2 = sums[2] / (NB * 3 * NPTS)
    loss = CHAMFER_WEIGHT * loss_ch + (1.0 - CHAMFER_WEIGHT) * loss_l2
    return (
        np.float32(loss),
        np.float32(loss_ch),
        np.float32(loss_l2),
        pc_est,
    )


def run_sharded(inputs, trace=False, **kw):
    nc = get_program()
    in_maps = make_in_maps(**inputs)
    res = run_bass_kernel_spmd(
        nc, in_maps, core_ids=list(range(NCORES)), trace=trace, **kw
    )
    return combine_outputs(res.results), res


def kernel(**inputs):
    out, _ = run_sharded(inputs, trace=False)
    return out


# revision 87
# speedup vs baseline: 1.0371x; 1.0322x over previous
"""DeepLatent loss kernel for Trainium2 (8 NeuronCores, data-parallel over batch).

Computes, for pc/pc_gt [64,3,1024], latent [64,64] and a 3-layer per-point MLP:
    noise  = MLP(concat(pc, latent))          (per-point 1x1 convs)
    pc_est = pc - noise
    loss_chamfer = bidirectional mean-min-sqdist(pc_gt, pc_est)
    loss_L2      = mean((pc_gt - pc_est)**2)
    loss = 0.1*loss_chamfer + 0.9*loss_L2
Returns (loss, loss_chamfer, loss_L2, pc_est) like the reference.

Sharding: batch dim 64 -> 8 cores x 8 clouds. Each core runs the full
MLP + chamfer for its 8 clouds; scalar partial sums are combined on host.

Key design points:
- All matmuls stream as float32r (TF32-class, 4x the fp32 rate). bf16 would
  be fatal for the distance matrix (|gt|^2+|est|^2-2gt.est cancels ~6-scale
  terms down to ~0.05 minima).
- The distance matrix is ONE K=9 matmul per 128-row tile: rows pair
  (gt, -2est), (gt^2, ones), (ones, est^2) so the PE also performs the
  |gt|^2 / |est|^2 summations.
- Per tile, a custom DVE op (COPY_MIN) turns the PSUM tile into a bf16
  SBUF copy AND the per-partition row-min in a single 1x pass; the
  opposite-direction running min then runs on bf16 SBUF pairs at DVE 2x.
- The final cross-partition min uses PE transposes + one DVE reduce.
- Emission is software-pipelined: batch b+1's MLP stages are emitted
  between batch b's chamfer tiles so the static per-engine schedule
  interleaves them.
"""

import sys

sys.path.insert(0, "/opt/trn_rl_repo")

import numpy as np

import concourse.bacc as bacc
import concourse.tile as tile
from concourse import dve_ops, mybir
from concourse.bass_utils import run_bass_kernel_spmd
from concourse.dve_spec import C0, Spec, Src0, lower, minn
from concourse.dve_uop import DveOpSpec

NB = 64          # total batch
NPTS = 1024      # points per cloud
LAT = 64         # latent dim
HID = 128        # MLP hidden
NCORES = 8
BPC = NB // NCORES          # clouds per core
FN = BPC * NPTS             # free-dim elements per core (8192)
CHAMFER_WEIGHT = 0.1

F32 = mybir.dt.float32
F32R = mybir.dt.float32r
BF16 = mybir.dt.bfloat16
AX = mybir.AxisListType
OP = mybir.AluOpType
ACTFN = mybir.ActivationFunctionType

_PROGRAM_CACHE = {}


def _register_copy_min():
    """Custom DVE op: out = copy(in0) (with dtype cast), accum_out = min(in0).

    One DVE pass turns a PSUM fp32 distance tile into a bf16 SBUF copy AND
    the per-partition row-min — replacing a reduce + copy pair. The micro-op
    program is packed into the per-NEFF DVE table at compile time.
    """
    name = "COPY_MIN_ANT"
    if name in dve_ops._SUB_OPCODE_FOR_NAME:
        for op in dve_ops.OPS:
            if op.name == name:
                return op

    def _ref(in0, in1, c0, c1, c2):
        b = np.asarray(in0, np.float32)
        return b, np.minimum(c0, b.reshape(b.shape[0], -1).min(axis=-1, keepdims=True))

    spec = Spec(body=Src0, accum=minn, accum_init=C0, reference=_ref)  # s0= +huge
    op = dve_ops.DveOp(name, spec, subdim=False, uops_sha={})
    dve_ops.OPS.append(op)
    dve_ops._SUB_OPCODE_FOR_NAME[name] = max(dve_ops._SUB_OPCODE_FOR_NAME.values()) + 1
    assert dve_ops._SUB_OPCODE_FOR_NAME[name] < 0x20
    dve_ops.CUSTOM_DVE_SPECS[name] = spec
    for ver in ("v3", "v4"):
        compiled = DveOpSpec(
            name=name,
            opcode=dve_ops.get_dve_sub_opcode(name),
            uops=lower(spec, ver=ver),
            rd1_en=False,
        )
        op.uops_sha[ver] = compiled.sha(ver)
    return op


COPY_MIN = _register_copy_min()


def build_program():
    """Build the per-core Bass/Tile program (same NEFF on all 8 cores)."""
    nc = bacc.Bacc("TRN2", target_bir_lowering=False, debug=False)

    # ---- I/O --------------------------------------------------------------
    # pgT:    rows 0-2 pc, 3-5 gt, 6 ones                      [7, FN]
    # gaT:    rows 0-2 gt, 3-5 zeros(->gt^2 via hop), 6-8 ones [9, FN]
    # eauginit: rows 0-2 any(->-2est), 3-5 ones, 6-8 any(->est^2)
    pgT = nc.declare_dram_parameter("pgT", [7, FN], F32R, isOutput=False)
    gaT = nc.declare_dram_parameter("gaT", [9, FN], F32R, isOutput=False)
    eauginit = nc.declare_dram_parameter("eauginit", [3, FN], F32R, isOutput=False)
    latT = nc.declare_dram_parameter("latT", [LAT, BPC], F32R, isOutput=False)
    w1pT = nc.declare_dram_parameter("w1pT", [3, HID], F32R, isOutput=False)
    w1lT = nc.declare_dram_parameter("w1lT", [LAT, HID], F32R, isOutput=False)
    b1c = nc.declare_dram_parameter("b1c", [HID, 1], F32, isOutput=False)
    w2T = nc.declare_dram_parameter("w2T", [HID, HID], F32R, isOutput=False)
    b2c = nc.declare_dram_parameter("b2c", [HID, 1], F32, isOutput=False)
    w3n6 = nc.declare_dram_parameter("w3n6", [HID, 36], F32R, isOutput=False)
    pgl = nc.declare_dram_parameter("pgl", [7, 36], F32R, isOutput=False)
    identin = nc.declare_dram_parameter("identin", [HID, HID], BF16, isOutput=False)
    ones128in = nc.declare_dram_parameter("ones128in", [HID, 1], F32, isOutput=False)
    zeros4in = nc.declare_dram_parameter("zeros4in", [HID, 4], F32, isOutput=False)

    est_out = nc.declare_dram_parameter("est", [3, FN], F32, isOutput=True)
    partials = nc.declare_dram_parameter("partials", [4, 1], F32, isOutput=True)

    with tile.TileContext(nc) as tc:
        with (
            tc.tile_pool(name="singles", bufs=1) as singles,
            tc.tile_pool(name="hpool", bufs=3) as hpool,
            tc.tile_pool(name="rpool", bufs=3) as rpool,
            tc.tile_pool(name="jpool", bufs=3) as jpool,
            tc.tile_pool(name="spool", bufs=3) as spool,
            tc.tile_pool(name="mmp", bufs=2, space="PSUM") as mmp,
            tc.tile_pool(name="dps", bufs=2, space="PSUM") as dps,
        ):
            # ---- constants / big persistent tiles -------------------------
            # (pg/ga/eaug are DMA'd in per-cloud slices so cloud 0's MLP and
            # chamfer deps clear early instead of waiting on 800KB of DMA)
            pg = singles.tile([7, FN], F32R)
            ga = singles.tile([9, FN], F32R)
            eaug = singles.tile([9, FN], F32R)
            est_sb = singles.tile([3, FN], F32)

            lat_sb = singles.tile([LAT, BPC], F32R)
            nc.sync.dma_start(out=lat_sb[:], in_=latT[:])
            w1p_sb = singles.tile([3, HID], F32R)
            nc.sync.dma_start(out=w1p_sb[:], in_=w1pT[:])
            w1l_sb = singles.tile([LAT, HID], F32R)
            nc.sync.dma_start(out=w1l_sb[:], in_=w1lT[:])
            b1_sb = singles.tile([HID, 1], F32)
            nc.sync.dma_start(out=b1_sb[:], in_=b1c[:])
            w2_sb = singles.tile([HID, HID], F32R)
            nc.sync.dma_start(out=w2_sb[:], in_=w2T[:])
            b2_sb = singles.tile([HID, 1], F32)
            nc.sync.dma_start(out=b2_sb[:], in_=b2c[:])
            w3_sb = singles.tile([HID, 36], F32R)
            nc.sync.dma_start(out=w3_sb[:], in_=w3n6[:])
            pgl_sb = singles.tile([7, 36], F32R)
            nc.sync.dma_start(out=pgl_sb[:], in_=pgl[:])
            ident = singles.tile([HID, HID], BF16)
            nc.sync.dma_start(out=ident[:], in_=identin[:])
            ones128 = singles.tile([HID, 1], F32)
            nc.sync.dma_start(out=ones128[:], in_=ones128in[:])

            # cloud 0's slices first so its MLP/chamfer deps clear early;
            # ga rows 3-5 and eaug rows 0-2/6-8 are device-written before
            # any read, so only the host-meaningful rows are DMA'd.
            s0 = slice(0, NPTS)
            with tc.high_priority():
                nc.sync.dma_start(out=pg[:, s0], in_=pgT[:, s0])
                nc.sync.dma_start(out=ga[0:3, s0], in_=gaT[0:3, s0])
                nc.sync.dma_start(out=ga[6:9, s0], in_=gaT[6:9, s0])
                nc.sync.dma_start(out=eaug[3:6, s0], in_=eauginit[:, s0])
            rs = slice(NPTS, FN)
            nc.sync.dma_start(out=pg[:, rs], in_=pgT[:, rs])
            nc.sync.dma_start(out=ga[0:3, rs], in_=gaT[0:3, rs])
            nc.sync.dma_start(out=ga[6:9, rs], in_=gaT[6:9, rs])
            nc.sync.dma_start(out=eaug[3:6, rs], in_=eauginit[:, rs])

            dirA = singles.tile([HID, BPC], F32)    # per-batch row-min sums
            dirB = singles.tile([HID, BPC], F32)    # per-batch col-min sums
            l2acc = singles.tile([3, BPC], F32)     # per-batch L2 partial sums

            # ---- W1 latent term: bias1[:, b] = W1[:,3:] @ latent[b] + b1 --
            w1lat_ps = mmp.tile([HID, NPTS], F32, tag="mm")
            nc.tensor.matmul(
                w1lat_ps[:, 0:BPC], w1l_sb[:], lat_sb[:], start=True, stop=True
            )
            bias1 = singles.tile([HID, BPC], F32)
            nc.vector.tensor_scalar_add(bias1[:], w1lat_ps[:, 0:BPC], b1_sb[:])

            # ---- per-cloud stage emitters ---------------------------------
            def emit_h1(b):
                b0 = b * NPTS
                h1_ps = mmp.tile([HID, NPTS], F32, tag="mm")
                nc.tensor.matmul(
                    h1_ps[:, 0:512], w1p_sb[:], pg[0:3, b0 : b0 + 512],
                    start=True, stop=True,
                )
                nc.tensor.matmul(
                    h1_ps[:, 512:NPTS], w1p_sb[:], pg[0:3, b0 + 512 : b0 + NPTS],
                    start=True, stop=True,
                )
                h1 = hpool.tile([HID, NPTS], F32R, tag="h1")
                with tc.high_priority(offset=48):
                    nc.scalar.activation(
                        h1[:], h1_ps[:], ACTFN.Relu, bias=bias1[:, b : b + 1], scale=1.0
                    )
                return h1

            def emit_h2(b, h1):
                h2_ps = mmp.tile([HID, NPTS], F32, tag="mm")
                nc.tensor.matmul(
                    h2_ps[:, 0:512], w2_sb[:], h1[:, 0:512], start=True, stop=True
                )
                nc.tensor.matmul(
                    h2_ps[:, 512:NPTS], w2_sb[:], h1[:, 512:NPTS], start=True, stop=True
                )
                h2 = hpool.tile([HID, NPTS], F32R, tag="h2")
                with tc.high_priority(offset=48):
                    nc.scalar.activation(
                        h2[:], h2_ps[:], ACTFN.Relu, bias=b2_sb[:], scale=1.0
                    )
                return h2

            def emit_est_mm(b, h2, half, ed_ps=None, after=None):
                # est/diff: psum rows 0-2 = pc_est, rows 32-34 = gt - pc_est
                # (engine APs may only start at partitions 0/32/64/96).
                # Halves are emitted separately so the PE work spreads over
                # two chamfer-tile gaps instead of blocking one d fill.
                b0 = b * NPTS
                if ed_ps is None:
                    ed_ps = mmp.tile([36, NPTS], F32, tag="mm")
                lo, hi = half * 512, (half + 1) * 512
                m1 = nc.tensor.matmul(
                    ed_ps[:, lo:hi], w3_sb[:], h2[:, lo:hi], start=True, stop=False
                )
                if after is not None:
                    tile.add_dep_helper(
                        m1.ins, after.ins, sync=False,
                        reason="keep ED pair behind the current d fill",
                    )
                nc.tensor.matmul(
                    ed_ps[:, lo:hi], pgl_sb[:], pg[:, b0 + lo : b0 + hi],
                    start=False, stop=True,
                )
                return ed_ps

            def emit_est_tail(b, ed_ps):
                b0 = b * NPTS
                bsl = slice(b0, b0 + NPTS)
                # chamfer-critical ops first (the scheduler otherwise parks
                # them behind the next cloud's ReLUs, delaying the d matmuls)
                with tc.high_priority(offset=64):
                    nc.scalar.mul(eaug[0:3, bsl], ed_ps[0:3, :], -2.0)
                    esq = jpool.tile([3, NPTS], F32R, tag="esq")
                    nc.scalar.activation(esq[:], ed_ps[0:3, :], ACTFN.Square)
                    # SBUF->SBUF DMA hop: engines cannot write partitions 6-8
                    nc.sync.dma_start(out=eaug[6:9, bsl], in_=esq[:])
                nc.scalar.copy(est_sb[:, bsl], ed_ps[0:3, :])
                junk = jpool.tile([3, NPTS], F32, tag="junk")
                nc.scalar.activation(
                    junk[:], ed_ps[32:35, :], ACTFN.Square,
                    accum_out=l2acc[:, b : b + 1],
                )
                nc.sync.dma_start(out=est_out[:, bsl], in_=est_sb[:, bsl])

            def emit_gtprep(b):
                # gt^2 rows for the augmented lhsT (rows 3-5 of ga, via hop)
                b0 = b * NPTS
                with tc.high_priority(offset=64):
                    gtsq = jpool.tile([3, NPTS], F32R, tag="gtsq")
                    nc.scalar.activation(
                        gtsq[:], ga[0:3, b0 : b0 + NPTS], ACTFN.Square
                    )
                    nc.sync.dma_start(out=ga[3:6, b0 : b0 + NPTS], in_=gtsq[:])

            def emit_mlp(b):
                h1 = emit_h1(b)
                h2 = emit_h2(b, h1)
                ed = emit_est_mm(b, h2, 0)
                emit_est_mm(b, h2, 1, ed)
                emit_est_tail(b, ed)
                emit_gtprep(b)

            def emit_colmin(b, rmin_final, rowmins):
                # col-min: transpose rmin 128x128 chunks, reduce over partitions
                t_ps = mmp.tile([HID, 8, HID], BF16, tag="mm")
                for u in range(8):
                    nc.tensor.transpose(
                        t_ps[:, u, :], rmin_final[:, u * HID : (u + 1) * HID], ident[:]
                    )
                colmins = spool.tile([HID, 8], F32, tag="colmins")
                nc.vector.tensor_reduce(
                    out=colmins[:], in_=t_ps[:], axis=AX.X, op=OP.min
                )
                nc.vector.tensor_reduce(
                    out=dirA[:, b : b + 1], in_=rowmins[:], axis=AX.X, op=OP.add
                )
                nc.vector.tensor_reduce(
                    out=dirB[:, b : b + 1], in_=colmins[:], axis=AX.X, op=OP.add
                )

            # ---- prologue: cloud 0's MLP + cloud 1's layer 1 --------------
            emit_mlp(0)
            h1_cache = {}
            if BPC > 1:
                h1_cache[1] = emit_h1(1)

            # ---- main software-pipelined loop -----------------------------
            pending = None  # (b, rmin_final, rowmins) awaiting colmin
            for b in range(BPC):
                b0 = b * NPTS
                rowmins = spool.tile([HID, 8], F32, tag="rowmins")
                rmin_prev = None
                nb = b + 1
                for t in range(8):
                    d_ps = dps.tile([HID, NPTS], F32, tag="d")
                    off = b0 + t * HID
                    nc.tensor.matmul(
                        d_ps[:, 0:512], ga[:, off : off + HID],
                        eaug[:, b0 : b0 + 512], start=True, stop=True,
                    )
                    dmm2 = nc.tensor.matmul(
                        d_ps[:, 512:NPTS], ga[:, off : off + HID],
                        eaug[:, b0 + 512 : b0 + NPTS], start=True, stop=True,
                    )
                    d_sb = rpool.tile([HID, NPTS], BF16, tag="dsb")
                    nc.vector._custom_dve(
                        COPY_MIN,
                        out=d_sb[:],
                        in0=d_ps[:],
                        s0=3.0e38,
                        accum_out=rowmins[:, t : t + 1],
                    )
                    if t == 0:
                        rmin_prev = d_sb
                        # previous cloud's colmin, after this cloud's first
                        # tile is in flight (keeps PE on d tiles, no DVE gap)
                        if pending is not None:
                            emit_colmin(*pending)
                            pending = None
                    else:
                        rmin = rpool.tile([HID, NPTS], BF16, tag="rmin")
                        nc.vector.tensor_tensor(
                            out=rmin[:], in0=d_sb[:], in1=rmin_prev[:], op=OP.min
                        )
                        rmin_prev = rmin
                    # two-batch-deep pipeline: every MLP stage of cloud
                    # b+1 has a full batch of lead over its in-order PE slot,
                    # so nothing stalls waiting on a ReLU semaphore
                    if nb < BPC:
                        if t == 0:
                            h2_next = emit_h2(nb, h1_cache.pop(nb))
                        elif t == 2:
                            ed_next = emit_est_mm(nb, h2_next, 0, after=dmm2)
                        elif t == 4:
                            emit_est_mm(nb, h2_next, 1, ed_next, after=dmm2)
                            emit_est_tail(nb, ed_next)
                        elif t == 5:
                            emit_gtprep(nb)
                        elif t == 6 and nb + 1 < BPC:
                            h1_cache[nb + 1] = emit_h1(nb + 1)
                pending = (b, rmin_prev, rowmins)
            emit_colmin(*pending)

            # ---- final partial sums (cross-partition via ones matmul) -----
            fin = singles.tile([HID, 4], F32)
            nc.sync.dma_start(out=fin[:], in_=zeros4in[:])
            nc.vector.tensor_reduce(out=fin[:, 0:1], in_=dirA[:], axis=AX.X, op=OP.add)
            nc.vector.tensor_reduce(out=fin[:, 1:2], in_=dirB[:], axis=AX.X, op=OP.add)
            nc.vector.tensor_reduce(
                out=fin[0:3, 2:3], in_=l2acc[:], axis=AX.X, op=OP.add
            )
            pp = mmp.tile([4, 1], F32, tag="mm")
            nc.tensor.matmul(pp[:], fin[:], ones128[:], start=True, stop=True)
            outp = singles.tile([4, 1], F32)
            nc.scalar.copy(outp[:], pp[:])
            nc.sync.dma_start(out=partials[:], in_=outp[:])

    nc.finalize()
    return nc


def get_program():
    if "nc" not in _PROGRAM_CACHE:
        _PROGRAM_CACHE["nc"] = build_program()
    return _PROGRAM_CACHE["nc"]


def make_in_maps(pc, pc_gt, latent, W1, b1, W2, b2, W3, b3):
    import ml_dtypes

    pc = np.ascontiguousarray(pc, dtype=np.float32)
    pc_gt = np.ascontiguousarray(pc_gt, dtype=np.float32)
    latent = np.ascontiguousarray(latent, dtype=np.float32)
    W1 = np.asarray(W1, dtype=np.float32)
    b1 = np.asarray(b1, dtype=np.float32)
    W2 = np.asarray(W2, dtype=np.float32)
    b2 = np.asarray(b2, dtype=np.float32)
    W3 = np.asarray(W3, dtype=np.float32)
    b3 = np.asarray(b3, dtype=np.float32)

    eye3 = np.eye(3, dtype=np.float32)
    pgl = np.zeros((7, 36), np.float32)
    pgl[0:3, 0:3] = eye3        # est rows: +pc
    pgl[0:3, 32:35] = -eye3     # diff rows: -pc
    pgl[3:6, 32:35] = eye3      # diff rows: +gt
    pgl[6, 0:3] = -b3           # est rows: -b3
    pgl[6, 32:35] = b3          # diff rows: +b3
    w3n36 = np.zeros((128, 36), np.float32)
    w3n36[:, 0:3] = -W3.T       # est rows: -(W3 @ h2)
    w3n36[:, 32:35] = W3.T      # diff rows: +(W3 @ h2)
    consts = {
        "w1pT": np.ascontiguousarray(W1[:, :3].T),
        "w1lT": np.ascontiguousarray(W1[:, 3:].T),
        "b1c": b1[:, None].copy(),
        "w2T": np.ascontiguousarray(W2.T),
        "b2c": b2[:, None].copy(),
        "w3n6": w3n36,
        "pgl": pgl,
        "identin": np.eye(HID, dtype=ml_dtypes.bfloat16),
        "ones128in": np.ones((HID, 1), np.float32),
        "zeros4in": np.zeros((HID, 4), np.float32),
    }

    ones1 = np.ones((1, FN), np.float32)
    eauginit = np.ones((3, FN), np.float32)
    in_maps = []
    for i in range(NCORES):
        sl = slice(i * BPC, (i + 1) * BPC)
        pcT = pc[sl].transpose(1, 0, 2).reshape(3, FN)
        gtT = pc_gt[sl].transpose(1, 0, 2).reshape(3, FN)
        pgT = np.concatenate([pcT, gtT, ones1], axis=0)
        gaT = np.concatenate(
            [gtT, np.zeros((3, FN), np.float32), np.ones((3, FN), np.float32)],
            axis=0,
        )
        latTc = np.ascontiguousarray(latent[sl].T)
        in_maps.append(
            {
                "pgT": np.ascontiguousarray(pgT),
                "gaT": np.ascontiguousarray(gaT),
                "eauginit": eauginit,
                "latT": latTc,
                **consts,
            }
        )
    return in_maps


def combine_outputs(results):
    """results: list (per core) of {"est": [3, FN], "partials": [4, 1]}."""
    est_parts = []
    sums = np.zeros(4, dtype=np.float64)
    for r in results:
        est_parts.append(
            np.asarray(r["est"]).reshape(3, BPC, NPTS).transpose(1, 0, 2)
        )
        sums += np.asarray(r["partials"], dtype=np.float64)[:, 0]
    pc_est = np.concatenate(est_parts, axis=0).astype(np.float32)
    loss_ch = sums[0] / (NB * NPTS) + sums[1] / (NB * NPTS)
    loss_l2 = sums[2] / (NB * 3 * NPTS)
    loss = CHAMFER_WEIGHT * loss_ch + (1.0 - CHAMFER_WEIGHT) * loss_l2
    return (
        np.float32(loss),
        np.float32(loss_ch),
        np.float32(loss_l2),
        pc_est,
    )


def run_sharded(inputs, trace=False, **kw):
    nc = get_program()
    in_maps = make_in_maps(**inputs)
    res = run_bass_kernel_spmd(
        nc, in_maps, core_ids=list(range(NCORES)), trace=trace, **kw
    )
    return combine_outputs(res.results), res


def kernel(**inputs):
    out, _ = run_sharded(inputs, trace=False)
    return out
